# revision 34
# baseline (speedup 1.0000x reference)
"""Trainium2 Bass kernel for nn_DynamicsShaper: time-varying RBJ lowpass biquad
driven by per-segment-averaged logits.

Sharding: batch row r -> NeuronCore r (8 rows, 8 cores, fully independent).

Per-core layout: the row of T=160000 samples is viewed as [128 partitions x
W=1250].  First-order recurrences (segmented cumsum for run means, reverse
hold-scan for broadcast) use the DVE TensorTensorScan instruction per
partition, chained across partitions via a PE transpose + a [.,128] scan.
The order-2 IIR uses a blocked scan: C=25 chunks of L=50 per partition run
three coupled recursions (zero-state response + two homogeneous solutions)
in lockstep, then chunk-to-chunk affine state maps are combined by a
3-basis walk within each partition and a log2(128)-round Hillis-Steele
(PE shift matrices) across partitions, followed by a linear correction pass.
"""

import sys

sys.path.insert(0, "/opt/trn_rl_repo")

import numpy as np

import concourse.bass as bass
import concourse.bacc as bacc
import concourse.mybir as mybir
import concourse.tile as tile
from concourse import masks

P = 128          # SBUF partitions
W = 1250         # samples per partition (T = P*W)
C = 25           # chunks per partition
L = W // C       # chunk length (50)
T = P * W
B = 8
SR = 16000.0
GAIN_MIN, GAIN_MAX = 0.1, 2.0
LOG_MIN_W = float(np.log(2.0 * np.pi * 20.0 / SR))
LOG_MAX_W = float(np.log(np.pi))
LOG_MIN_Q, LOG_MAX_Q = float(np.log(0.0707)), float(np.log(2.0))

fp = mybir.dt.float32
i32 = mybir.dt.int32
OP = mybir.AluOpType
AF = mybir.ActivationFunctionType


def _act_recip(nc, out, in_, bias=0.0, scale=1.0):
    """ACT-table reciprocal 1/(scale*x + bias); refine with Newton after.
    (bass's activation() helper refuses Reciprocal; build the instruction
    directly -- we always follow with a Newton step on DVE.)"""
    eng = nc.scalar
    inputs = [
        eng.lower_ap(in_),
        mybir.ImmediateValue(dtype=mybir.dt.float32, value=float(bias)),
        mybir.ImmediateValue(dtype=mybir.dt.float32, value=float(scale)),
        mybir.ImmediateValue(dtype=mybir.dt.float32, value=0.0),
    ]
    return eng.add_instruction(
        mybir.InstActivation(
            name=nc.get_next_instruction_name(),
            func=AF.Reciprocal,
            ins=inputs,
            outs=[eng.lower_ap(out)],
        )
    )


DEBUG_TAPS = False


def build_program():
    nc = bacc.Bacc("TRN2", target_bir_lowering=False, debug=False, num_devices=B)
    d_noise = nc.dram_tensor("noise", [P, W], fp, kind="ExternalInput").ap()
    d_seg = nc.dram_tensor("seg", [P, W], i32, kind="ExternalInput").ap()
    d_logits = nc.dram_tensor("logits", [P, 3 * W], fp, kind="ExternalInput").ap()
    d_bnd = nc.dram_tensor("bnd", [P, 2], fp, kind="ExternalInput").ap()
    d_y = nc.dram_tensor("y", [P, W], fp, kind="ExternalOutput").ap()
    taps = {}
    if DEBUG_TAPS:
        def tap(name, ap):
            t = nc.dram_tensor(f"dbg_{name}", list(ap.shape), ap.dtype,
                               kind="ExternalOutput").ap()
            nc.sync.dma_start(t, ap)
            taps[name] = t
    else:
        def tap(name, ap):
            pass
    with tile.TileContext(nc) as tc:
        _body(nc, tc, d_noise, d_seg, d_logits, d_bnd, d_y, tap)
    nc.compile()
    return nc


def _body(nc, tc, d_noise, d_seg, d_logits, d_bnd, d_y, tap=lambda n, a: None):
    from contextlib import ExitStack
    ctx = ExitStack()
    pool = ctx.enter_context(tc.tile_pool(name="main", bufs=1))
    psum = ctx.enter_context(tc.tile_pool(name="ps", bufs=1, space="PSUM"))

    V = nc.vector
    G = nc.gpsimd
    A = nc.scalar

    # ---------- loads (seg first: it gates the first compute) ----------
    seg = pool.tile([P, W], i32)
    logits = pool.tile([P, 3 * W], fp)
    noise = pool.tile([P, W], fp)
    cmp = pool.tile([P, W + 1], fp)
    nc.sync.dma_start(seg[:, 0:W // 2], d_seg[:, 0:W // 2])
    nc.sync.dma_start(seg[:, W // 2:W], d_seg[:, W // 2:W])
    nc.sync.dma_start(cmp[:, 0:1], d_bnd[:, 0:1])
    nc.sync.dma_start(cmp[:, W:W + 1], d_bnd[:, 1:2])
    for c in (1, 2):
        nc.sync.dma_start(logits[:, c * W:(c + 1) * W],
                          d_logits[:, c * W:(c + 1) * W])
    nc.sync.dma_start(logits[:, 0:W], d_logits[:, 0:W])
    nc.sync.dma_start(noise[:], d_noise)

    # ---------- constants: identity + shift matrices ----------
    ident = pool.tile([P, P], fp)
    masks.make_identity(nc, ident[:])
    ident8 = pool.tile([8, 8], fp)
    masks.make_identity(nc, ident8[:])

    zmat = pool.tile([P, P], fp)
    G.memset(zmat[:], 0.0)

    def shift_mat(base):
        m = pool.tile([P, P], fp, name=f"shift_{base}")
        G.affine_select(out=m[:], in_=zmat[:], compare_op=OP.not_equal, fill=1.0,
                        base=base, pattern=[[-1, P]], channel_multiplier=1)
        return m

    sh_up = {s: shift_mat(s) for s in (1, 2, 4, 8, 16, 32, 64)}  # out[p] = in[p-s]

    # identity-affine pads for HS rounds: rows < s get identity map
    # map layout per 6 cols: (d1, p1, q1, d2, p2, q2); identity: p1=1, q2=1
    idpad = {}
    for s in (1, 2, 4, 8, 16, 32, 64):
        t = pool.tile([P, 6], fp, name=f"idpad_{s}")
        V.memset(t[:], 0.0)
        V.memset(t[0:s, 1:2], 1.0)
        V.memset(t[0:s, 5:6], 1.0)
        idpad[s] = t

    # small scalar-bias constants (set up while input DMA is in flight)
    one1 = pool.tile([P, 1], fp)
    V.memset(one1[:], 1.0)
    bias_w = pool.tile([P, 1], fp)
    V.memset(bias_w[:], LOG_MIN_W)
    bias_q = pool.tile([P, 1], fp)
    V.memset(bias_q[:], -LOG_MIN_Q - float(np.log(2.0)))  # folds alpha's 0.5
    bias_hp = pool.tile([P, 1], fp)
    V.memset(bias_hp[:], float(np.pi / 2))
    two_b = pool.tile([P, 1], fp)
    V.memset(two_b[:], 2.0)
    half_b = pool.tile([P, 1], fp)
    V.memset(half_b[:], 0.5)



    # ---------- gates ----------
    # cmp[P, W+1]: cmp[:, j] (1<=j<=W-1) = (seg[j] == seg[j-1]); col 0 = gate
    # at partition start; col W = "continues into next partition".  The two
    # boundary columns are host-computed (d_bnd) since they need cross-
    # partition neighbors.  Two halves so the first starts as soon as the
    # first half of seg lands.
    Wh = W // 2
    V.tensor_tensor(cmp[:, 1:Wh], seg[:, 1:Wh], seg[:, :Wh - 1], OP.is_equal)
    V.tensor_tensor(cmp[:, Wh:W], seg[:, Wh:], seg[:, Wh - 1:W - 1], OP.is_equal)
    g = cmp[:, 0:W]
    e = cmp[:, 1:W + 1]
    # whole-partition gate products collapse to (seg[0]==seg[W-1])*boundary
    sameseg = pool.tile([P, 1], fp)
    V.tensor_tensor(sameseg[:], seg[:, 0:1], seg[:, W - 1:W], OP.is_equal)
    gPc = pool.tile([P, 1], fp)
    V.tensor_tensor(gPc[:], sameseg[:], cmp[:, 0:1], OP.mult)
    gRc = pool.tile([P, 1], fp)
    V.tensor_tensor(gRc[:], sameseg[:], cmp[:, W:W + 1], OP.mult)
    # full-width gate prefix/suffix products (Pool lacks is_equal; keep on
    # DVE, scheduled into the chain PE-wait gaps)
    Gp = pool.tile([P, W], fp)    # prefix product of gates EXCLUDING g[p,0]
    Erev = pool.tile([P, W], fp)  # suffix product of e EXCLUDING e[p, W-1]

    # ---------- forward segmented scans (zero init), ch1 first ----------
    d0 = [pool.tile([P, W], fp, name=f"d0_{c}") for c in range(3)]
    l0 = pool.tile([P, W], fp)
    # ACT warm-up ops that depend only on the gates / raw noise
    ie = pool.tile([P, W], fp)
    A.activation(ie[:], e, AF.Identity, scale=-1.0,
                 bias=nc.const_aps.tensor(1.0, (P, 1)))  # 1-e
    f16 = mybir.dt.float16
    noise16 = pool.tile([P, W], f16)
    A.activation(noise16[:], noise[:], AF.Copy)
    V.tensor_tensor_scan(l0[:], g, one1[:].to_broadcast([P, W]), 0.0,
                         OP.mult, OP.add)
    V.tensor_tensor_scan(d0[1][:], g, logits[:, W:2 * W], 0.0,
                         OP.mult, OP.add)

    # ---------- cross-partition chain helpers ----------
    def chain_fwd(tails, tag):
        """Exclusive chain over partitions for forward scans.  tails: list of
        [P,1] tail APs.  Returns dIn [P, n] (incoming prefix per channel)."""
        n = len(tails)
        sA = pool.tile([P, 2 * n], fp, name=f"s_{tag}")
        V.tensor_copy(sA[:, 0:n], gPc[:].to_broadcast([P, n]))
        for i, tl in enumerate(tails):
            V.tensor_copy(sA[:, n + i:n + i + 1], tl)
        pg = psum.tile([4, P], fp, tag="pg_ch")
        pd = psum.tile([4, P], fp, tag="pd_ch")
        nc.tensor.transpose(pg[0:n, :], sA[:, 0:n], ident[:])
        nc.tensor.transpose(pd[0:n, :], sA[:, n:2 * n], ident[:])
        tg = pool.tile([n, P], fp, name=f"tg_{tag}")
        td = pool.tile([n, P], fp, name=f"td_{tag}")
        V.tensor_copy(tg[:], pg[0:n, :])
        V.tensor_copy(td[:], pd[0:n, :])
        chv = pool.tile([n, P], fp, name=f"chv_{tag}")
        V.tensor_tensor_scan(chv[:], tg[:], td[:], 0.0, OP.mult, OP.add)
        shv = pool.tile([n, P], fp, name=f"shv_{tag}")
        V.memset(shv[:, 0:1], 0.0)
        V.tensor_copy(shv[:, 1:P], chv[:, 0:P - 1])
        pc = psum.tile([P, 4], fp, tag="pc_ch")
        nc.tensor.matmul(pc[:, 0:n], shv[:], ident8[0:n, 0:n])
        dv = pool.tile([P, n], fp, name=f"dIn_{tag}")
        V.tensor_scalar_mul(dv[:], pc[:, 0:n], cmp[:, 0:1])
        return dv

    def chain_rev_ph1(heads, tag):
        """Phase 1: stage heads + PE transposes.  Returns state for ph2/ph3."""
        n = len(heads)
        sA = pool.tile([P, 2 * n], fp, name=f"s_{tag}")
        V.tensor_copy(sA[:, 0:n], gRc[:].to_broadcast([P, n]))
        for i, hd in enumerate(heads):
            V.tensor_copy(sA[:, n + i:n + i + 1], hd)
        pg = psum.tile([4, P], fp, tag="pg_rv")
        pd = psum.tile([4, P], fp, tag="pd_rv")
        nc.tensor.transpose(pg[0:n, :], sA[:, 0:n], ident[:])
        nc.tensor.transpose(pd[0:n, :], sA[:, n:2 * n], ident[:])
        return (n, tag, pg, pd)

    def chain_rev_ph2(st):
        """Phase 2: chain scan + shift + PE matmul back."""
        n, tag, pg, pd = st
        tg = pool.tile([n, P], fp, name=f"tg_{tag}")
        td = pool.tile([n, P], fp, name=f"td_{tag}")
        V.tensor_copy(tg[:], pg[0:n, :])
        V.tensor_copy(td[:], pd[0:n, :])
        chv = pool.tile([n, P], fp, name=f"chv_{tag}")
        V.tensor_tensor_scan(chv[:, ::-1], tg[:, ::-1], td[:, ::-1],
                             0.0, OP.mult, OP.add)
        shv = pool.tile([n, P], fp, name=f"shv_{tag}")
        V.memset(shv[:, P - 1:P], 0.0)
        V.tensor_copy(shv[:, 0:P - 1], chv[:, 1:P])
        pc = psum.tile([P, 4], fp, tag="pc_rv")
        nc.tensor.matmul(pc[:, 0:n], shv[:], ident8[0:n, 0:n])
        return (n, tag, pc)

    def chain_rev_ph3(st):
        n, tag, pc = st
        dv = pool.tile([P, n], fp, name=f"mIn_{tag}")
        V.tensor_scalar_mul(dv[:], pc[:, 0:n], cmp[:, W:W + 1])
        return dv

    # ---------- pipelines: chainA(l,d1) after 2 scans, chainB(d2,d0) after 4
    d = [logits[:, c * W:(c + 1) * W] for c in range(3)]
    m = d
    dInA = chain_fwd([l0[:, W - 1:W], d0[1][:, W - 1:W]], "fa")
    V.tensor_tensor_scan(d0[2][:], g, logits[:, 2 * W:3 * W], 0.0,
                         OP.mult, OP.add)
    V.tensor_tensor(Gp[:], seg[:], seg[:, 0:1].to_broadcast([P, W]),
                    OP.is_equal)
    V.tensor_tensor_scan(d0[0][:], g, logits[:, 0:W], 0.0, OP.mult, OP.add)
    V.tensor_tensor(Erev[:], seg[:], seg[:, W - 1:W].to_broadcast([P, W]),
                    OP.is_equal)
    dInB = chain_fwd([d0[2][:, W - 1:W], d0[0][:, W - 1:W]], "fb")
    l = pool.tile([P, W], fp)
    V.scalar_tensor_tensor(l[:], Gp[:], dInA[:, 0:1], l0[:], OP.mult, OP.add)
    V.scalar_tensor_tensor(d[1], Gp[:], dInA[:, 1:2], d0[1][:], OP.mult, OP.add)
    rl = d0[1]  # dead after d1 correction
    _act_recip(nc, rl[:], l[:])  # table recip; error only reaches run means
    V.scalar_tensor_tensor(d[2], Gp[:], dInB[:, 0:1], d0[2][:], OP.mult, OP.add)
    h = l0  # dead after l
    V.tensor_tensor(h[:], ie[:], rl[:], OP.mult)
    dat = [pool.tile([P, W], fp, name=f"dat_{c}") for c in range(3)]
    m0 = [pool.tile([P, W], fp, name=f"m0_{c}") for c in range(3)]
    V.tensor_tensor(dat[1][:], d[1][:], h[:], OP.mult)
    V.tensor_tensor_scan(m0[1][:, ::-1], e[:, ::-1], dat[1][:, ::-1],
                         0.0, OP.mult, OP.add)
    r1 = chain_rev_ph1([m0[1][:, 0:1]], "r1")
    V.tensor_tensor(dat[2][:], d[2][:], h[:], OP.mult)
    r1b = chain_rev_ph2(r1)
    V.tensor_tensor_scan(m0[2][:, ::-1], e[:, ::-1], dat[2][:, ::-1],
                         0.0, OP.mult, OP.add)
    mIn1 = chain_rev_ph3(r1b)
    V.scalar_tensor_tensor(m[1], Erev[:], mIn1[:, 0:1], m0[1][:],
                           OP.mult, OP.add)
    r2 = chain_rev_ph1([m0[2][:, 0:1]], "r2")
    V.scalar_tensor_tensor(d[0], Gp[:], dInB[:, 1:2], d0[0][:], OP.mult, OP.add)
    r2b = chain_rev_ph2(r2)
    V.tensor_tensor(dat[0][:], d[0][:], h[:], OP.mult)
    mIn2 = chain_rev_ph3(r2b)
    V.scalar_tensor_tensor(m[2], Erev[:], mIn2[:, 0:1], m0[2][:],
                           OP.mult, OP.add)
    V.tensor_tensor_scan(m0[0][:, ::-1], e[:, ::-1], dat[0][:, ::-1],
                         0.0, OP.mult, OP.add)
    rb = chain_rev_ph1([m0[0][:, 0:1]], "rb")
    rbb = chain_rev_ph2(rb)
    mInB = chain_rev_ph3(rbb)
    V.scalar_tensor_tensor(m[0], Erev[:], mInB[:, 0:1], m0[0][:],
                           OP.mult, OP.add)

    tap("m0c", m[0])
    tap("m1c", m[1])
    tap("m2c", m[2])
    # ---------- ACT chain: sigmoids grouped, then exps, then sin/cos ------
    sg1h = pool.tile([P, W], f16)
    sg2h = pool.tile([P, W], f16)
    sg0h = pool.tile([P, W], f16)
    A.activation(sg1h[:], m[1][:], AF.Sigmoid)
    A.activation(sg2h[:], m[2][:], AF.Sigmoid)
    w = d0[1]  # dead after d1 correction
    A.activation(w[:], sg1h[:], AF.Exp, bias=bias_w[:],
                 scale=(LOG_MAX_W - LOG_MIN_W))
    qinvh = pool.tile([P, W], f16)   # 1/(2q)
    A.activation(qinvh[:], sg2h[:], AF.Exp, bias=bias_q[:],
                 scale=-(LOG_MAX_Q - LOG_MIN_Q))
    sinwh = pool.tile([P, W], f16)
    A.activation(sinwh[:], w[:], AF.Sin)
    cosw = l0  # dead after h
    A.activation(cosw[:], w[:], AF.Sin, bias=bias_hp[:], scale=-1.0)
    A.activation(sg0h[:], m[0][:], AF.Sigmoid)

    # ---------- gain / x / FIR path (fp16; fills the ACT exp/sin window) ---
    gain16 = sg2h  # dead after qinvh
    V.tensor_scalar(gain16[:], sg0h[:], GAIN_MAX - GAIN_MIN, GAIN_MIN,
                    OP.mult, OP.add)
    x = pool.tile([P, W], f16)
    V.tensor_tensor(x[:], noise16[:], gain16[:], OP.mult)
    xt32 = pool.tile([P, 2], fp)
    V.tensor_copy(xt32[:], x[:, W - 2:W])
    ps_x = psum.tile([P, 2], fp, tag="ps_small")
    nc.tensor.matmul(ps_x[:], sh_up[1][:], xt32[:])
    xb = pool.tile([P, 2], fp)   # (x[p-1, W-2], x[p-1, W-1]); row0 = 0
    V.tensor_copy(xb[:], ps_x[:])
    x2t = sg1h     # dead after w
    V.tensor_scalar(x2t[:], x[:], 2.0, 0.0, OP.mult, OP.add)
    s_f = pool.tile([P, W], f16)
    f_t = pool.tile([P, W], f16)
    V.tensor_tensor(s_f[:, 2:], x2t[:, 1:W - 1], x[:, 2:], OP.add)
    V.tensor_tensor(f_t[:, 2:], s_f[:, 2:], x[:, :W - 2], OP.add)
    V.scalar_tensor_tensor(s_f[:, 0:1], xb[:, 1:2], 2.0, x[:, 0:1],
                           OP.mult, OP.add)
    V.tensor_tensor(f_t[:, 0:1], s_f[:, 0:1], xb[:, 0:1], OP.add)
    V.scalar_tensor_tensor(s_f[:, 1:2], x[:, 0:1], 2.0, x[:, 1:2],
                           OP.mult, OP.add)
    V.tensor_tensor(f_t[:, 1:2], s_f[:, 1:2], xb[:, 1:2], OP.add)

    # ---------- remaining biquad coefficients (Newton all on DVE) ----------
    alphah = pool.tile([P, W], f16)  # alpha = sin(w)/(2q)
    V.tensor_tensor(alphah[:], sinwh[:], qinvh[:], OP.mult)
    r0a = m0[1]  # dead after m1 correction
    _act_recip(nc, r0a[:], alphah[:], bias=1.0)              # ~1/(1+alpha)
    nsc2 = d0[2]
    V.scalar_tensor_tensor(nsc2[:], alphah[:], 1.0, r0a[:],
                           OP.add, OP.mult)                  # (1+alpha)*r0
    nsc3 = dat[1]  # dead after m0[1] scan
    V.tensor_scalar(nsc3[:], nsc2[:], -1.0, 2.0, OP.mult, OP.add)
    inva0 = cmp  # dead after chains
    V.tensor_tensor(inva0[:, 0:W], nsc3[:], r0a[:], OP.mult)
    b0pre = dat[2]  # dead after m0[2] scan
    A.activation(b0pre[:], cosw[:], AF.Identity, scale=-0.5, bias=half_b[:])
    na1 = ie      # dead after h
    V.scalar_tensor_tensor(na1[:], cosw[:], 2.0, inva0[:, 0:W], OP.mult, OP.mult)
    na2 = dat[0]  # dead after m0[0] scan
    V.scalar_tensor_tensor(na2[:], alphah[:], 1.0, inva0[:, 0:W],
                           OP.subtract, OP.mult)
    b016 = pool.tile([P, W], f16)
    V.tensor_tensor(b016[:], b0pre[:], inva0[:, 0:W], OP.mult)
    tap("inva0", inva0[:, 0:W])
    tap("na1", na1[:])
    tap("na2", na2[:])
    fsc = pool.tile([P, W], f16)
    V.tensor_tensor(fsc[:], f_t[:], b016[:], OP.mult)
    f = fsc

    # ---------- double-step composite coefficients ----------
    # pair m covers steps n=2m, n=2m+1:
    #   v_n     = na1_n v_{n-1} + na2_n v_{n-2} (+ f_n)
    #   v_{n+1} = A_m  v_{n-1} + B_m  v_{n-2} (+ F_m)
    # with A = na1_{n+1} na1_n + na2_{n+1}, B = na1_{n+1} na2_n,
    #      F = na1_{n+1} f_n + f_{n+1}.
    Lh = L // 2
    na13 = na1.rearrange("p (c n) -> p c n", c=C)
    na23 = na2.rearrange("p (c n) -> p c n", c=C)
    f3 = f.rearrange("p (c n) -> p c n", c=C)
    n1e = na13[:, :, 0:L:2]
    n1o = na13[:, :, 1:L:2]
    n2e = na23[:, :, 0:L:2]
    n2o = na23[:, :, 1:L:2]
    Amt = pool.tile([P, C * Lh], fp)
    Amt3 = Amt.rearrange("p (c m) -> p c m", c=C)
    V.tensor_tensor(Amt3[:], n1o, n1e, OP.mult)
    Amf = pool.tile([P, C * Lh], fp)
    Amf3 = Amf.rearrange("p (c m) -> p c m", c=C)
    V.tensor_tensor(Amf3[:], Amt3[:], n2o, OP.add)
    fD = pool.tile([P, C * Lh * 2], fp)
    fD4 = fD.rearrange("p (c m k) -> p c m k", c=C, m=Lh, k=2)
    coefD = pool.tile([P, C * Lh * 12], fp)
    cD4 = coefD.rearrange("p (c m k) -> p c m k", c=C, m=Lh, k=12)
    # Build coefficients for the first MS pair-steps first so the inner loop
    # can launch while the tail (m >= MS) is still being written.
    MS = 6
    for mlo, mhi in ((0, MS), (MS, Lh)):
        fe = f3[:, :, 2 * mlo:L:2] if mhi == Lh else f3[:, :, 2 * mlo:2 * mhi:2]
        fo = f3[:, :, 2 * mlo + 1:L:2] if mhi == Lh else \
            f3[:, :, 2 * mlo + 1:2 * mhi:2]
        n1o_s = na13[:, :, 2 * mlo + 1:L:2] if mhi == Lh else \
            na13[:, :, 2 * mlo + 1:2 * mhi:2]
        V.tensor_tensor(fD4[:, :, mlo:mhi, 1:2], n1o_s.unsqueeze(3),
                        fe.unsqueeze(3), OP.mult)
        V.tensor_tensor(fD4[:, :, mlo:mhi, 1:2], fD4[:, :, mlo:mhi, 1:2],
                        fo.unsqueeze(3), OP.add)
        V.tensor_copy(fD4[:, :, mlo:mhi, 0:1], fe.unsqueeze(3))
        nm = mhi - mlo
        A.activation(cD4[:, :, mlo:mhi, 0:3],
                     n2e[:, :, mlo:mhi].unsqueeze(3).to_broadcast(
                         [P, C, nm, 3]), AF.Copy)
        A.activation(cD4[:, :, mlo:mhi, 3:6],
                     n1e[:, :, mlo:mhi].unsqueeze(3).to_broadcast(
                         [P, C, nm, 3]), AF.Copy)
        V.tensor_tensor(cD4[:, :, mlo:mhi, 6:9],
                        n1o[:, :, mlo:mhi].unsqueeze(3).to_broadcast(
                            [P, C, nm, 3]),
                        n2e[:, :, mlo:mhi].unsqueeze(3).to_broadcast(
                            [P, C, nm, 3]), OP.mult)
        A.activation(cD4[:, :, mlo:mhi, 9:12],
                     Amf3[:, :, mlo:mhi].unsqueeze(3).to_broadcast(
                         [P, C, nm, 3]), AF.Copy)

    # ---------- within-chunk recursions (y_zs, p, q interleaved) ----------
    # Chunks are split across DVE (0..CD-1) and Pool (CD..C-1): two
    # independent serial chains running concurrently.
    # ypqX[P, cn, (L+2)*3]: slot k holds 3 values (y, p, q) for recursion
    # index k-2; slots 0,1 are the initial conditions.
    CD = C
    ypqA = pool.tile([P, CD * (L + 2) * 3], fp)
    ypqA3 = ypqA.rearrange("p (c m) -> p c m", c=CD)
    V.memset(ypqA3[:, :, 0:6], 0.0)
    V.memset(ypqA3[:, :, 2:3], 1.0)   # q_{-2} = 1
    V.memset(ypqA3[:, :, 4:5], 1.0)   # p_{-1} = 1
    uA = pool.tile([P, CD * 12], fp)
    uA4 = uA.rearrange("p (c s k) -> p c s k", c=CD, s=2, k=6)
    parts = ((V, ypqA3, uA4, 0, CD),)
    for m in range(Lh):
        n = 2 * m
        for eng, y3t, u4t, lo, hi in parts:
            cn = hi - lo
            prevs = y3t[:, :, 3 * n:3 * n + 6].unsqueeze(2).to_broadcast(
                [P, cn, 2, 6])
            coefv = cD4[:, lo:hi, m, :].rearrange("p c (s k) -> p c s k",
                                                  s=2, k=6)
            eng.tensor_tensor(u4t[:], prevs, coefv, OP.mult)
            eng.tensor_tensor(
                y3t[:, :, 3 * n + 6:3 * n + 12].rearrange(
                    "p c (s k) -> p c s k", s=2, k=3),
                u4t[:, :, :, 0:3], u4t[:, :, :, 3:6], OP.add)
            eng.tensor_tensor(y3t[:, :, 3 * n + 6:3 * n + 10:3],
                              y3t[:, :, 3 * n + 6:3 * n + 10:3],
                              fD4[:, lo:hi, m, :], OP.add)

    tap("f", f[:])
    tap("coefD", coefD[:])
    tap("fD", fD[:])
    tap("ypqA", ypqA[:])
    # ---------- chunk-map prefix composition (log rounds along chunks) ----
    # Chunk map layout (d1,p1,q1,d2,p2,q2): row1 = chunk-out alpha' (v_{L-1})
    # as affine fn of the chunk-in state (alpha, beta); row2 = v_{L-2}.
    # Hillis-Steele inclusive prefix along the chunk (free) dim: shifts are
    # plain view offsets, no PE needed.
    base = 3 * L
    # prefix scratch lives in l (dead after rl): 150+150+300+150 = 750 floats
    mpa = l[:, 0:C * 6]
    mpb = l[:, C * 6:2 * C * 6]
    ut = l[:, 2 * C * 6:2 * C * 6 + C * 12]
    vt = l[:, 2 * C * 6 + C * 12:2 * C * 6 + C * 12 + C * 6]
    mpa3 = mpa.rearrange("p (c k) -> p c k", c=C)
    for y3t, lo, hi in ((ypqA3, 0, CD),):
        src = y3t[:, :, base:base + 6].rearrange(
            "p c (r k) -> p c r k", r=2, k=3)[:, :, ::-1, :]
        V.tensor_copy(mpa3[:, lo:hi, :].rearrange(
            "p c (r k) -> p c r k", r=2, k=3), src)
    cur, new = mpa, mpb
    s = 1
    while s < C:
        act = C - s
        c3 = cur.rearrange("p (c k) -> p c k", c=C)
        n3 = new.rearrange("p (c k) -> p c k", c=C)
        v3 = vt.rearrange("p (c k) -> p c k", c=C)
        u3 = ut.rearrange("p (c m) -> p c m", c=C)
        arows = c3[:, 0:act, :].rearrange("p c (k j) -> p c k j", k=2, j=3)
        for r in range(2):
            u4v = u3[:, 0:act, 6 * r:6 * r + 6].rearrange(
                "p c (k j) -> p c k j", k=2, j=3)
            bco = c3[:, s:C, 3 * r + 1:3 * r + 3].unsqueeze(3).to_broadcast(
                [P, act, 2, 3])
            V.tensor_tensor(u4v, bco, arows, OP.mult)
            V.tensor_tensor(v3[:, 0:act, 3 * r:3 * r + 3],
                            u3[:, 0:act, 6 * r:6 * r + 3],
                            u3[:, 0:act, 6 * r + 3:6 * r + 6], OP.add)
        V.tensor_tensor(n3[:, s:C, 0:4:3], v3[:, 0:act, 0:4:3],
                        c3[:, s:C, 0:4:3], OP.add)
        V.tensor_copy(
            n3[:, s:C, :].rearrange("p c (r k) -> p c r k", r=2, k=3)[
                :, :, :, 1:3],
            v3[:, 0:act, :].rearrange("p c (r k) -> p c r k", r=2, k=3)[
                :, :, :, 1:3])
        V.tensor_copy(n3[:, 0:s, :], c3[:, 0:s, :])
        cur, new = new, cur
        s *= 2
    cur3 = cur.rearrange("p (c k) -> p c k", c=C)
    # total partition map feeds the cross-partition scan
    Mcur = pool.tile([P, 6], fp)
    V.tensor_copy(Mcur[:], cur3[:, C - 1, :])

    # ---------- Hillis-Steele inclusive scan of affine maps over partitions ----
    Mnew = pool.tile([P, 6], fp)
    ash = pool.tile([P, 6], fp)
    v6 = pool.tile([P, 6], fp)
    u1t = pool.tile([P, 12], fp)
    u2t = pool.tile([P, 6], fp)
    ps_m = psum.tile([P, 6], fp)
    idmap = pool.tile([P, 6], fp)
    V.memset(idmap[:], 0.0)
    V.memset(idmap[:, 1:2], 1.0)
    V.memset(idmap[:, 5:6], 1.0)
    cur, new = Mcur, Mnew
    for s in (1, 2, 4, 8, 16, 32, 64):
        nc.tensor.matmul(ps_m[:], sh_up[s][:], cur[:])
        V.tensor_tensor(ash[:], ps_m[:], idpad[s][:], OP.add)
        bd = cur[:, 0:4:3].unsqueeze(2)                    # [P, 2, 1]
        # fused: u[r, t, k] = a_group[t][k] * b_scalar[r][t]
        a4 = ash.rearrange("p (t k) -> p t k", t=2).unsqueeze(1).to_broadcast(
            [P, 2, 2, 3])
        b4 = cur.rearrange("p (r k) -> p r k", r=2)[:, :, 1:3].unsqueeze(
            3).to_broadcast([P, 2, 2, 3])
        u1 = u1t.rearrange("p (r t k) -> p r t k", r=2, t=2)
        v = v6.rearrange("p (r k) -> p r k", r=2)
        nw = new.rearrange("p (r k) -> p r k", r=2)
        V.tensor_tensor(u1[:], a4, b4, OP.mult)
        V.tensor_tensor(v[:], u1[:, :, 0, :], u1[:, :, 1, :], OP.add)
        V.tensor_tensor(nw[:, :, 0:1], v[:, :, 0:1], bd, OP.add)
        V.tensor_copy(nw[:, :, 1:3], v[:, :, 1:3])
        cur, new = new, cur
    # exclusive d: alpha0/beta0 per partition = d-cols of T^hat_{p-1}
    ps_d = psum.tile([P, 2], fp, tag="ps_small")
    nc.tensor.matmul(ps_d[:], sh_up[1][:], cur[:, 0:4:3])
    ab0 = pool.tile([P, 2], fp)   # (alpha0, beta0)
    V.tensor_copy(ab0[:], ps_d[:])

    # ---------- per-chunk incoming states from prefix + (alpha0, beta0) ----
    # chunk 0 sees (alpha0, beta0); chunk c>0 sees the exclusive prefix
    # cur3[:, c-1, :] applied to (alpha0, beta0) -- two stt ops per row.
    alc = pool.tile([P, C], fp)
    bec = pool.tile([P, C], fp)
    tq = pool.tile([P, C - 1], fp)
    V.tensor_copy(alc[:, 0:1], ab0[:, 0:1])
    V.tensor_copy(bec[:, 0:1], ab0[:, 1:2])
    ex3 = cur3[:, 0:C - 1, :]

    def excol(col):
        return ex3[:, :, col:col + 1].rearrange("p c k -> p (c k)")

    for dst, r in ((alc, 0), (bec, 1)):
        V.scalar_tensor_tensor(tq[:], excol(3 * r + 1), ab0[:, 0:1],
                               excol(3 * r), OP.mult, OP.add)
        V.scalar_tensor_tensor(dst[:, 1:C], excol(3 * r + 2), ab0[:, 1:2],
                               tq[:], OP.mult, OP.add)

    # ---------- correction pass: y = y_zs + p*alpha_c + q*beta_c ----------
    yfinA = noise  # dead after noise16 cast
    t1A = Gp       # dead after d[0] correction
    t2A = Erev     # dead after m[0] correction
    Cq = C // 4
    fparts = ((V, 0, Cq), (V, Cq, 2 * Cq), (V, 2 * Cq, 3 * Cq), (V, 3 * Cq, C))
    for eng, lo, hi in fparts:
        cn = hi - lo
        y3t = ypqA3[:, lo:hi, :]
        pv = y3t[:, :, 7:6 + 3 * L:3]
        qv = y3t[:, :, 8:6 + 3 * L:3]
        yzs = y3t[:, :, 6:4 + 3 * L:3]
        alcv = alc[:, lo:hi].unsqueeze(2).to_broadcast([P, cn, L])
        becv = bec[:, lo:hi].unsqueeze(2).to_broadcast([P, cn, L])
        y3o = yfinA.rearrange("p (c n) -> p c n", c=C)[:, lo:hi, :]
        t13 = t1A.rearrange("p (c n) -> p c n", c=C)[:, lo:hi, :]
        t23 = t2A.rearrange("p (c n) -> p c n", c=C)[:, lo:hi, :]
        eng.tensor_tensor(t13[:], pv, alcv, OP.mult)
        eng.tensor_tensor(t23[:], qv, becv, OP.mult)
        eng.tensor_tensor(y3o[:], t13[:], yzs, OP.add)
        eng.tensor_tensor(y3o[:], y3o[:], t23[:], OP.add)
        nc.sync.dma_start(d_y[:, lo * L:hi * L], yfinA[:, lo * L:hi * L])


_NC_CACHE = None


def _get_nc():
    global _NC_CACHE
    if _NC_CACHE is None:
        _NC_CACHE = build_program()
    return _NC_CACHE


def make_in_maps(noise, seg, lg):
    maps = []
    for r in range(len(noise)):
        s2 = seg[r].reshape(P, W)
        bnd = np.zeros((P, 2), np.float32)
        bnd[1:, 0] = (s2[1:, 0] == s2[:-1, W - 1])
        bnd[:-1, 1] = (s2[1:, 0] == s2[:-1, W - 1])
        maps.append({
            "noise": noise[r].reshape(P, W),
            "seg": s2,
            "logits": np.concatenate(
                [lg[r, :, c].reshape(P, W) for c in range(3)], axis=1),
            "bnd": bnd,
        })
    return maps


def kernel(noise_bursts, segment_ids, logits):
    from concourse.bass_utils import run_bass_kernel_spmd

    noise = np.ascontiguousarray(np.asarray(noise_bursts, dtype=np.float32))
    seg = np.ascontiguousarray(np.asarray(segment_ids).astype(np.int32))
    lg = np.ascontiguousarray(np.asarray(logits, dtype=np.float32))
    assert noise.shape == (B, T) and seg.shape == (B, T) and lg.shape == (B, T, 3)

    nc = _get_nc()
    in_maps = make_in_maps(noise, seg, lg)
    res = run_bass_kernel_spmd(nc, in_maps, list(range(B)))
    out = np.stack([res.results[r]["y"].reshape(T) for r in range(B)])
    return out.astype(np.float32)

